# revision 1
# baseline (speedup 1.0000x reference)
"""Trainium2 Bass kernel for a Griffin-style ChimeraBlock:
   pre-norm RG-LRU recurrence branch + pre-norm SwiGLU FFN, B=2, T=2048,
   D=H=2048, FFN=5632, fp32 I/O.

Parallelization over 8 NeuronCores (tensor-parallel):
  - recurrence hidden dim H sharded 8x (256 per core); scan is elementwise
    per channel -> shards cleanly (native DVE tensor_tensor_scan op).
  - norm1 stats computed LOCALLY per core from the replicated bf16 x
    (square + ones-matmul partition reduction) -> no collective on the
    critical path into the recurrence.
  - AllGather of hs (bf16, raw) -> rec_out computed column-parallel with
    the rmsnorm scale applied post-matmul (per-column), gated on a tiny
    AllReduce of partial sum-of-squares that overlaps the matmuls.
  - xnew = x + rec_out AllGathered raw (bf16); norm2's scale is likewise
    applied post-matmul inside the FFN, so the AllReduce for its stats
    stays off the matmul critical path.
  - FFN hidden sharded 8x (704 -> padded 768 per core); down-proj partials
    ReduceScattered per 512-column chunk; each core emits its D-shard.
All weights are prefetched into SBUF at kernel start. Matmuls run in bf16
(fp32 accumulation in PSUM); gate/scan state in fp32; residual adds in
fp32. Host side only reshapes/transposes/casts/shards and folds the
(all-ones) rmsnorm gain vectors into adjacent weight matrices.
"""

import sys

sys.path.insert(0, "/opt/trn_rl_repo")

import numpy as np
import ml_dtypes

import concourse.bass as bass
import concourse.mybir as mybir
import concourse.tile as tile
from concourse import bacc
from concourse.bass_utils import run_bass_kernel_spmd

BF16 = mybir.dt.bfloat16
F32 = mybir.dt.float32
AF = mybir.ActivationFunctionType
OP = mybir.AluOpType

B, T, D = 2, 2048, 2048
H, FFN = 2048, 5632
NC = 8
HS = H // NC          # 256 hidden shard
DS = D // NC          # 256 d-model shard (output sharding)
FS = FFN // NC        # 704 ffn shard
FSP = 768             # ffn shard padded to a multiple of 128 (pad weights = 0)
BT = B * T            # 4096
CH = 512              # time-chunk (columns)
NCH = BT // CH        # 8 chunks
CPB = T // CH         # 4 chunks per batch element (scan resets at b boundary)
KD = D // 128         # 16 k-tiles when contracting over D
KH = H // 128         # 16 k-tiles when contracting over H
KF = FSP // 128       # 6 k-tiles when contracting over ffn shard
EPS = 1e-6
CCONST = 8.0

NP_BF16 = ml_dtypes.bfloat16


def _r128(ap):
    # [R, N] dram view -> [128, R//128, N] (partition, k-tile, col)
    return ap.rearrange("(k p) n -> p k n", p=128)


def build_nc():
    nc = bacc.Bacc("TRN2", target_bir_lowering=False, debug=False, num_devices=NC)
    rg = [list(range(NC))]

    # ---------------- kernel I/O (per core) ----------------
    xt = nc.dram_tensor("xt", [D, BT], BF16, kind="ExternalInput")      # x^T replicated
    xf32 = nc.dram_tensor("xf32", [DS, BT], F32, kind="ExternalInput")  # f32 x^T d-shard
    w3 = nc.dram_tensor("w3", [D, 3 * HS], BF16, kind="ExternalInput")  # in|ig|rg lhsT shard
    wro = nc.dram_tensor("wro", [H, DS], BF16, kind="ExternalInput")    # rec_out lhsT d-shard
    wg = nc.dram_tensor("wg", [D, FSP], BF16, kind="ExternalInput")
    wu = nc.dram_tensor("wu", [D, FSP], BF16, kind="ExternalInput")
    wd = nc.dram_tensor("wd", [FSP, D], BF16, kind="ExternalInput")
    # cols: 0 = rec_lambda, 1 = ig bias, 2 = rg bias, 3 = h0
    smalls = nc.dram_tensor("smalls", [HS, 4], F32, kind="ExternalInput")
    y = nc.dram_tensor("y", [DS, BT], F32, kind="ExternalOutput")

    with tile.TileContext(nc) as tc:
        with (
            tc.tile_pool(name="sb", bufs=2) as sb,
            tc.tile_pool(name="ps", bufs=2, space="PSUM") as ps,
            tc.tile_pool(name="dr", bufs=1, space="DRAM") as dr,
        ):
            build_body(nc, tc, sb, ps, dr, rg,
                       xt, xf32, w3, wro, wg, wu, wd, smalls, y)
    nc.compile()
    return nc


def build_body(nc, tc, sb, ps, dr, rg, xt, xf32, w3, wro, wg, wu, wd, smalls, y):
    AG = "AllGather"
    AR = "AllReduce"
    RS = "ReduceScatter"

    # ---------------- internal DRAM ----------------
    ar2_in = dr.tile([1, BT], F32, name="ar2_in")
    ar2_out = dr.tile([1, BT], F32, name="ar2_out", addr_space="Shared")
    ar3_in = dr.tile([1, BT], F32, name="ar3_in")
    ar3_out = dr.tile([1, BT], F32, name="ar3_out", addr_space="Shared")
    agin_hs = [dr.tile([HS, 2 * CH], BF16, name=f"agin_hs{j}") for j in range(4)]
    agout_hs = [dr.tile([H, 2 * CH], BF16, name=f"agout_hs{j}", addr_space="Shared")
                for j in range(4)]
    agin_h2 = [dr.tile([DS, 2 * CH], BF16, name=f"agin_h2{j}") for j in range(4)]
    agout_h2 = [dr.tile([D, 2 * CH], BF16, name=f"agout_h2{j}", addr_space="Shared")
                for j in range(4)]
    ffn_part = [dr.tile([D, CH], BF16, name=f"ffn_part{c}") for c in range(NCH)]
    ffn_red = [dr.tile([DS, CH], BF16, name=f"ffn_red{c}") for c in range(NCH)]

    dma = nc.sync.dma_start
    mm = nc.tensor.matmul

    # ---------------- constants / small tensors ----------------
    ones_bf = sb.tile([128, 1], BF16, name="ones_bf", tag="ones", bufs=1)
    nc.vector.memset(ones_bf[:], 1.0)
    ones_row = sb.tile([1, 128], BF16, name="ones_row", tag="onesr", bufs=1)
    nc.vector.memset(ones_row[:], 1.0)

    def const_tile(val, cname):
        t = sb.tile([128, 1], F32, name=cname, tag=cname, bufs=1)
        nc.vector.memset(t[:], val)
        return t

    c_ln8 = const_tile(1e-8, "c_ln8")         # Ln bias
    c_eps = const_tile(EPS, "c_eps")          # rmsnorm eps
    c_1eps = const_tile(1.0 + EPS, "c_1eps")  # 1 + eps for sqrt(1 - a^2 + eps)

    smalls_sb = sb.tile([128, 2, 4], F32, name="smalls_sb", tag="smalls", bufs=1)
    dma(out=smalls_sb[:], in_=smalls[:].rearrange("(a p) c -> p a c", p=128))
    sig_l = sb.tile([128, 2], F32, name="sig_l", tag="sig_l", bufs=1)
    nc.scalar.activation(sig_l[:], smalls_sb[:, :, 0], AF.Sigmoid)
    c8_sb = sb.tile([128, 2], F32, name="c8_sb", tag="c8", bufs=1)
    nc.scalar.activation(c8_sb[:], sig_l[:], AF.Ln, bias=c_ln8[:])
    nc.scalar.activation(c8_sb[:], c8_sb[:], AF.Copy, bias=0.0, scale=CCONST)

    # ---------------- weights: prefetch up front ----------------
    # chunk 0's input first (PE starts on its sumsq), then in-proj weights,
    # then the rest; FFN weights follow (needed much later).
    xc_tiles = {}
    xc_tiles[0] = sb.tile([128, KD, CH], BF16, name="xc0", tag="stream", bufs=2)
    dma(out=xc_tiles[0][:], in_=_r128(xt[:])[:, :, 0:CH])
    w3_sb = sb.tile([128, KD, 3 * HS], BF16, name="w3_sb", tag="wbig", bufs=1)
    dma(out=w3_sb[:], in_=_r128(w3[:]))
    xc_tiles[1] = sb.tile([128, KD, CH], BF16, name="xc1", tag="stream", bufs=2)
    dma(out=xc_tiles[1][:], in_=_r128(xt[:])[:, :, CH:2 * CH])
    wro_sb = sb.tile([128, KH, DS], BF16, name="wro_sb", tag="wro", bufs=1)
    dma(out=wro_sb[:], in_=_r128(wro[:]))
    wg_sb = sb.tile([128, KD, FSP], BF16, name="wg_sb", tag="wg", bufs=1)
    dma(out=wg_sb[:], in_=_r128(wg[:]))
    wu_sb = sb.tile([128, KD, FSP], BF16, name="wu_sb", tag="wu", bufs=1)
    dma(out=wu_sb[:], in_=_r128(wu[:]))

    def arc_chain(cname, src_ap, scale):
        # inv rms = exp(-0.5 * ln(mean + eps)): two Scalar-engine table ops
        # (vector.reciprocal on a [1, CH] tile costs ~4.3us of serial DVE)
        arc = sb.tile([1, CH], F32, name=f"arc_{cname}", tag="row1", bufs=3)
        nc.scalar.activation(arc[:], src_ap, AF.Ln, bias=c_eps[:1, :],
                             scale=scale)
        arcb = sb.tile([1, CH], BF16, name=f"arcb_{cname}", tag="row1b", bufs=3)
        nc.scalar.activation(arcb[:], arc[:], AF.Exp, bias=0.0, scale=-0.5)
        return arcb

    def bcast_mm(cname, arcb):
        # broadcast [1, CH] -> [128, CH] with a rank-1 matmul + PSUM copy;
        # keeps the GpSimd queue free for collective triggers (a
        # partition_broadcast there serializes against them).
        pbc = ps.tile([128, CH], F32, name=f"pbc_{cname}", tag="psq", bufs=2)
        mm(pbc[:], ones_row[:], arcb[:], start=True, stop=True)
        invc = sb.tile([128, CH], F32, name=f"invc_{cname}", tag="invcf", bufs=3)
        nc.scalar.copy(invc[:], pbc[:])
        return invc

    # ======== phase 2: local norm1 + in-proj + gates + scan ========
    hst_prev = None
    deferred_psq2 = []   # (c, hsq) for chunks 6/7: emitted after P4's head

    def emit_psq2(c, hsq):
        psq2 = ps.tile([1, CH], F32, name=f"psq2_{c}", tag="psq", bufs=2)
        mm(psq2[:], ones_bf[:], hsq[:, 0, :], start=True, stop=False)
        mm(psq2[:], ones_bf[:], hsq[:, 1, :], start=False, stop=True)
        sqs2 = sb.tile([1, CH], F32, name=f"sqs2_{c}", tag="row1", bufs=3)
        nc.scalar.copy(sqs2[:], psq2[:])
        dma(out=ar2_in[0:1, c * CH:(c + 1) * CH], in_=sqs2[:])

    for c in range(NCH):
        cs = slice(c * CH, (c + 1) * CH)
        j, jj = c // 2, c % 2

        if c in xc_tiles:
            xc = xc_tiles[c]
        else:
            xc = sb.tile([128, KD, CH], BF16, name=f"xc{c}", tag="stream", bufs=2)
            dma(out=xc[:], in_=_r128(xt[:])[:, :, cs])

        # local sum-of-squares over all of D (x is replicated in bf16)
        psq1 = ps.tile([1, CH], F32, name=f"psq1_{c}", tag="psq", bufs=2)
        for q in range(KD // 2):
            xsqq = sb.tile([128, 2, CH], BF16, name=f"xsqq{c}_{q}", tag="sq3d",
                           bufs=3)
            nc.vector.tensor_tensor(xsqq[:], xc[:, 2 * q:2 * q + 2, :],
                                    xc[:, 2 * q:2 * q + 2, :], op=OP.mult)
            mm(psq1[:], ones_bf[:], xsqq[:, 0, :],
               start=(q == 0), stop=False)
            mm(psq1[:], ones_bf[:], xsqq[:, 1, :],
               start=False, stop=(q == KD // 2 - 1))
        arcb1 = arc_chain(f"1_{c}", psq1[:], 1.0 / D)

        zt = {}
        invc = None
        for m in range(2):
            for p_i in range(3):  # 0: x_proj, 1: input gate, 2: recurrence gate
                pst = ps.tile([128, CH], F32, name=f"pp{c}_{p_i}_{m}", tag="mm",
                              bufs=6)
                for k in range(KD):
                    mm(pst[:],
                       w3_sb[:, k, p_i * HS + m * 128: p_i * HS + (m + 1) * 128],
                       xc[:, k, :],
                       start=(k == 0), stop=(k == KD - 1))
                if invc is None:
                    # emitted after one unit of matmuls: the ACT chain has
                    # finished by then, so the PE slots this in gap-free
                    invc = bcast_mm(f"1_{c}", arcb1)
                z = sb.tile([128, CH], BF16, name=f"z{c}_{p_i}_{m}", tag="z",
                            bufs=6)
                nc.vector.tensor_tensor(z[:], pst[:], invc[:], op=OP.mult)
                zt[(p_i, m)] = z

        hst = sb.tile([128, 2, CH], BF16, name=f"hst{c}", tag="hs", bufs=3)
        for m in range(2):
            zx, zi, zr = zt[(0, m)], zt[(1, m)], zt[(2, m)]
            # gate chain kept on the Scalar engine as one run of ACT ops --
            # cross-engine ping-pong latency dominates the chunk tail here:
            #   la = C*log_a*sigmoid(zr);  a = exp(la);  a^2 = exp(2*la);
            #   sq = sqrt((1+eps) - a^2)   [Sqrt with scale=-1, bias=1+eps]
            it = sb.tile([128, CH], BF16, name=f"it{c}_{m}", tag="it", bufs=2)
            nc.scalar.activation(it[:], zi[:], AF.Sigmoid,
                                 bias=smalls_sb[:, m, 1:2])
            rt = sb.tile([128, CH], F32, name=f"rt{c}_{m}", tag="rtna", bufs=3)
            nc.scalar.activation(rt[:], zr[:], AF.Sigmoid,
                                 bias=smalls_sb[:, m, 2:3])
            nc.scalar.activation(rt[:], rt[:], AF.Copy,
                                 scale=c8_sb[:, m:m + 1])
            at = sb.tile([128, CH], F32, name=f"at{c}_{m}", tag="at", bufs=2)
            nc.scalar.activation(at[:], rt[:], AF.Exp)
            e2 = sb.tile([128, CH], F32, name=f"e2_{c}_{m}", tag="rtna",
                         bufs=3)
            nc.scalar.activation(e2[:], rt[:], AF.Exp, scale=2.0)
            nc.scalar.activation(e2[:], e2[:], AF.Sqrt, bias=c_1eps[:],
                                 scale=-1.0)
            nc.vector.tensor_tensor(zx[:], it[:], zx[:], op=OP.mult)
            nc.vector.tensor_tensor(zx[:], e2[:], zx[:], op=OP.mult)
            if c % CPB == 0:
                init = smalls_sb[:, m, 3:4]
            else:
                init = hst_prev[:, m, CH - 1:CH]
            nc.vector.tensor_tensor_scan(hst[:, m, :], at[:], zx[:], init,
                                         op0=OP.mult, op1=OP.add)
        hst_prev = hst

        hsq = sb.tile([128, 2, CH], BF16, name=f"hsq{c}", tag="sq3d", bufs=3)
        nc.vector.tensor_tensor(hsq[:], hst[:], hst[:], op=OP.mult)
        if c >= 6:
            deferred_psq2.append((c, hsq))
        else:
            emit_psq2(c, hsq)
        dma(out=_r128(agin_hs[j][:])[:, :, jj * CH:(jj + 1) * CH], in_=hst[:])
        if jj == 1 and j < 3:
            nc.gpsimd.collective_compute(AG, OP.bypass, replica_groups=rg,
                                         ins=[agin_hs[j][:]], outs=[agout_hs[j][:]])

    # ======== phase 4: rec_out + residual + norm2 stats ========
    # Matmuls for the first two chunks are emitted BEFORE the deferred P2
    # stats and the AR2 collective, so the PE stays busy while the (DVE-
    # lagging) scan tail of chunks 6/7 finishes and AR2 flies.
    def p4_matmuls(c):
        jj = c % 2
        psts = [ps.tile([128, CH], F32, name=f"pro{c}_{m}", tag="mm", bufs=6)
                for m in range(2)]
        for h in range(2):
            hstm = sb.tile([128, KH // 2, CH], BF16, name=f"hstm{c}_{h}",
                           tag="hstm", bufs=2)
            dma(out=hstm[:],
                in_=_r128(agout_hs[c // 2][:])[:, h * (KH // 2):(h + 1) * (KH // 2),
                                               jj * CH:(jj + 1) * CH])
            for m_i in range(2):
                for k in range(KH // 2):
                    mm(psts[m_i][:],
                       wro_sb[:, h * (KH // 2) + k, m_i * 128:(m_i + 1) * 128],
                       hstm[:, k, :],
                       start=(h == 0 and k == 0),
                       stop=(h == 1 and k == KH // 2 - 1))
        return psts

    deferred_psq3 = []   # (c, xnq) for chunks 6/7: emitted after P6's head

    def p4_drains(c, psts):
        cs = slice(c * CH, (c + 1) * CH)
        j, jj = c // 2, c % 2
        arc2 = sb.tile([1, CH], F32, name=f"arc2_{c}", tag="row1", bufs=3)
        dma(out=arc2[:], in_=ar2_out[0:1, cs])
        nc.scalar.activation(arc2[:], arc2[:], AF.Ln, bias=c_eps[:1, :],
                             scale=1.0 / H)
        arcb2 = sb.tile([1, CH], BF16, name=f"arcb2_{c}", tag="row1b", bufs=3)
        nc.scalar.activation(arcb2[:], arc2[:], AF.Exp, bias=0.0, scale=-0.5)
        invc2 = bcast_mm(f"2_{c}", arcb2)
        xnt = sb.tile([128, 2, CH], BF16, name=f"xnt{c}", tag="xnt", bufs=2)
        for m_i in range(2):
            nc.vector.tensor_tensor(psts[m_i][:], psts[m_i][:], invc2[:],
                                    op=OP.mult)
            xft = sb.tile([128, CH], F32, name=f"xft{c}_{m_i}", tag="xf",
                          bufs=2)
            dma(out=xft[:], in_=_r128(xf32[:])[:, m_i, cs])
            nc.vector.tensor_tensor(xnt[:, m_i, :], psts[m_i][:], xft[:],
                                    op=OP.add)
        xnq = sb.tile([128, 2, CH], BF16, name=f"xnq{c}", tag="sq3d", bufs=3)
        nc.vector.tensor_tensor(xnq[:], xnt[:], xnt[:], op=OP.mult)
        if c >= 6:
            deferred_psq3.append((c, xnq))
        else:
            psq3 = ps.tile([1, CH], F32, name=f"psq3_{c}", tag="psq", bufs=2)
            mm(psq3[:], ones_bf[:], xnq[:, 0, :], start=True, stop=False)
            mm(psq3[:], ones_bf[:], xnq[:, 1, :], start=False, stop=True)
            sqs3 = sb.tile([1, CH], F32, name=f"sqs3_{c}", tag="row1", bufs=3)
            nc.scalar.copy(sqs3[:], psq3[:])
            dma(out=ar3_in[0:1, cs], in_=sqs3[:])
        dma(out=_r128(agin_h2[j][:])[:, :, jj * CH:(jj + 1) * CH], in_=xnt[:])
        if jj == 1 and j < 3:
            nc.gpsimd.collective_compute(AG, OP.bypass, replica_groups=rg,
                                         ins=[agin_h2[j][:]],
                                         outs=[agout_h2[j][:]])

    def emit_ar3_tail():
        for c, xnq in deferred_psq3:
            psq3 = ps.tile([1, CH], F32, name=f"psq3_{c}", tag="psq", bufs=2)
            mm(psq3[:], ones_bf[:], xnq[:, 0, :], start=True, stop=False)
            mm(psq3[:], ones_bf[:], xnq[:, 1, :], start=False, stop=True)
            sqs3 = sb.tile([1, CH], F32, name=f"sqs3_{c}", tag="row1", bufs=3)
            nc.scalar.copy(sqs3[:], psq3[:])
            dma(out=ar3_in[0:1, c * CH:(c + 1) * CH], in_=sqs3[:])
        nc.gpsimd.collective_compute(AR, OP.add, replica_groups=rg,
                                     ins=[ar3_in[:]], outs=[ar3_out[:]])

    held = {c: p4_matmuls(c) for c in (0, 1, 2)}
    for c, hsq in deferred_psq2:
        emit_psq2(c, hsq)
    # AR2 goes to the collective queue BEFORE the last (non-urgent) hs
    # AllGather so its result isn't stuck behind a 1MB transfer.
    nc.gpsimd.collective_compute(AR, OP.add, replica_groups=rg,
                                 ins=[ar2_in[:]], outs=[ar2_out[:]])
    nc.gpsimd.collective_compute(AG, OP.bypass, replica_groups=rg,
                                 ins=[agin_hs[3][:]], outs=[agout_hs[3][:]])
    # down-proj weights go into w3's (now free) slot; DMA overlaps P4
    wd_sb = sb.tile([128, KF, D], BF16, name="wd_sb", tag="wbig", bufs=1)
    dma(out=wd_sb[:], in_=_r128(wd[:]))

    for c in (0, 1, 2):
        p4_drains(c, held[c])
    for c in range(3, NCH):
        psts = p4_matmuls(c)
        p4_drains(c, psts)
    # AR3 ahead of the last h2 AllGather (same reasoning as AR2 above);
    # the deferred chunk-6/7 stats matmuls feed it.
    emit_ar3_tail()
    nc.gpsimd.collective_compute(AG, OP.bypass, replica_groups=rg,
                                 ins=[agin_h2[3][:]], outs=[agout_h2[3][:]])

    # ======== phase 6: FFN on raw gathered xnew ========
    for c in range(NCH):
        j, jj = c // 2, c % 2
        h2s = sb.tile([128, KD, CH], BF16, name=f"h2s{c}", tag="stream", bufs=2)
        dma(out=h2s[:], in_=_r128(agout_h2[j][:])[:, :, jj * CH:(jj + 1) * CH])
        gu = sb.tile([128, KF, CH], BF16, name=f"gu{c}", tag="gu", bufs=1)
        invc3 = None
        arcb3 = None
        for m_t in range(KF):
            psg = ps.tile([128, CH], F32, name=f"pg{c}_{m_t}", tag="mm", bufs=6)
            for k in range(KD):
                mm(psg[:], wg_sb[:, k, m_t * 128:(m_t + 1) * 128],
                   h2s[:, k, :], start=(k == 0), stop=(k == KD - 1))
            psu = ps.tile([128, CH], F32, name=f"pu{c}_{m_t}", tag="mm", bufs=6)
            for k in range(KD):
                mm(psu[:], wu_sb[:, k, m_t * 128:(m_t + 1) * 128],
                   h2s[:, k, :], start=(k == 0), stop=(k == KD - 1))
            if m_t == 0:
                arc3 = sb.tile([1, CH], F32, name=f"arc3_{c}", tag="row1",
                               bufs=3)
                dma(out=arc3[:], in_=ar3_out[0:1, c * CH:(c + 1) * CH])
                nc.scalar.activation(arc3[:], arc3[:], AF.Ln,
                                     bias=c_eps[:1, :], scale=1.0 / D)
                arcb3 = sb.tile([1, CH], BF16, name=f"arcb3_{c}", tag="row1b",
                                bufs=3)
                nc.scalar.activation(arcb3[:], arc3[:], AF.Exp, bias=0.0,
                                     scale=-0.5)
            # AR3-gated: for the first chunks give the PE two units of
            # matmuls to chew on before it hits this broadcast matmul
            if (c > 1 and m_t == 0) or (c <= 1 and m_t == 1):
                invc3 = bcast_mm(f"3_{c}", arcb3)

            def drain_unit(mu, psg_u, psu_u):
                t1 = sb.tile([128, CH], BF16, name=f"t1_{c}_{mu}", tag="bf1",
                             bufs=6)
                nc.vector.tensor_tensor(t1[:], psg_u[:], invc3[:], op=OP.mult)
                gs = sb.tile([128, CH], BF16, name=f"gs{c}_{mu}", tag="bf1",
                             bufs=6)
                nc.scalar.activation(gs[:], t1[:], AF.Silu)
                v = sb.tile([128, CH], BF16, name=f"v{c}_{mu}", tag="bf1",
                            bufs=6)
                nc.vector.tensor_tensor(v[:], psu_u[:], invc3[:], op=OP.mult)
                nc.vector.tensor_tensor(gu[:, mu, :], gs[:], v[:], op=OP.mult)

            if c <= 1 and m_t == 0:
                held_unit = (psg, psu)
            else:
                if c <= 1 and m_t == 1:
                    drain_unit(0, *held_unit)
                drain_unit(m_t, psg, psu)
        # down-proj: stage 4 m-tiles per SBUF tile -> 4 big DMAs per chunk
        for m4 in range(KD // 4):
            dst4 = sb.tile([128, 4, CH], BF16, name=f"dst{c}_{m4}", tag="dst4",
                           bufs=2)
            for mi in range(4):
                m_t = m4 * 4 + mi
                psd = ps.tile([128, CH], F32, name=f"pd{c}_{m_t}", tag="mm",
                              bufs=6)
                for k in range(KF):
                    mm(psd[:], wd_sb[:, k, m_t * 128:(m_t + 1) * 128],
                       gu[:, k, :], start=(k == 0), stop=(k == KF - 1))
                nc.scalar.copy(dst4[:, mi, :], psd[:])
            dma(out=ffn_part[c][m4 * 512:(m4 + 1) * 512, :]
                .rearrange("(a p) n -> p a n", p=128),
                in_=dst4[:])
        nc.gpsimd.collective_compute(RS, OP.add, replica_groups=rg,
                                     ins=[ffn_part[c][:]], outs=[ffn_red[c][:]])

    # ======== phase 7: final residual ========
    for c in range(NCH):
        cs = slice(c * CH, (c + 1) * CH)
        j, jj = c // 2, c % 2
        for m in range(2):
            frt = sb.tile([128, CH], BF16, name=f"frt{c}_{m}", tag="bf1", bufs=6)
            dma(out=frt[:], in_=_r128(ffn_red[c][:])[:, m, :])
            xb = sb.tile([128, CH], BF16, name=f"xb{c}_{m}", tag="bf1", bufs=6)
            dma(out=xb[:],
                in_=_r128(agin_h2[j][:])[:, m, jj * CH:(jj + 1) * CH])
            yt = sb.tile([128, CH], F32, name=f"yt{c}_{m}", tag="yt", bufs=2)
            nc.vector.tensor_tensor(yt[:], xb[:], frt[:], op=OP.add)
            dma(out=_r128(y[:])[:, m, cs], in_=yt[:])


_CACHE = {}


def _prep_inputs(inputs):
    f = np.float32
    x = np.asarray(inputs["x"], f)                       # [B, T, D]
    norm1_w = np.asarray(inputs["norm1_w"], f)
    rec_in_w = np.asarray(inputs["rec_in_w"], f)         # [H, D]
    rec_ig_w = np.asarray(inputs["rec_ig_w"], f)
    rec_ig_b = np.asarray(inputs["rec_ig_b"], f)
    rec_rg_w = np.asarray(inputs["rec_rg_w"], f)
    rec_rg_b = np.asarray(inputs["rec_rg_b"], f)
    rec_lambda = np.asarray(inputs["rec_lambda"], f)
    rec_out_w = np.asarray(inputs["rec_out_w"], f)       # [D, H]
    rec_h0 = np.asarray(inputs["rec_h0"], f)             # [1, 1, H]
    rec_norm_w = np.asarray(inputs["rec_norm_w"], f)
    norm2_w = np.asarray(inputs["norm2_w"], f)
    ffn_gate_w = np.asarray(inputs["ffn_gate_w"], f)     # [FFN, D]
    ffn_up_w = np.asarray(inputs["ffn_up_w"], f)
    ffn_down_w = np.asarray(inputs["ffn_down_w"], f)     # [D, FFN]

    xt_full = np.ascontiguousarray(
        x.reshape(BT, D).T.astype(NP_BF16))              # [D, BT]
    xt_f32 = np.ascontiguousarray(x.reshape(BT, D).T)    # [D, BT] f32

    # fold norm gains into adjacent weights; transpose into lhsT layouts
    w_in_t = (rec_in_w * norm1_w[None, :]).T             # [D, H]
    w_ig_t = (rec_ig_w * norm1_w[None, :]).T
    w_rg_t = (rec_rg_w * norm1_w[None, :]).T
    w_ro_t = (rec_out_w * rec_norm_w[None, :]).T         # [H, D]
    w_g_t = (ffn_gate_w * norm2_w[None, :]).T            # [D, FFN]
    w_u_t = (ffn_up_w * norm2_w[None, :]).T
    w_d_t = ffn_down_w.T                                 # [FFN, D]

    in_maps = []
    for r in range(NC):
        hsl = slice(r * HS, (r + 1) * HS)
        dsl = slice(r * DS, (r + 1) * DS)
        fsl = slice(r * FS, (r + 1) * FS)
        w3_r = np.concatenate(
            [w_in_t[:, hsl], w_ig_t[:, hsl], w_rg_t[:, hsl]], axis=1)
        wg_r = np.zeros((D, FSP), f)
        wg_r[:, :FS] = w_g_t[:, fsl]
        wu_r = np.zeros((D, FSP), f)
        wu_r[:, :FS] = w_u_t[:, fsl]
        wd_r = np.zeros((FSP, D), f)
        wd_r[:FS, :] = w_d_t[fsl, :]
        smalls_r = np.stack(
            [rec_lambda[hsl], rec_ig_b[hsl], rec_rg_b[hsl],
             np.broadcast_to(rec_h0[0, 0], (H,))[hsl]], axis=1)
        in_maps.append({
            "xt": xt_full,
            "xf32": np.ascontiguousarray(xt_f32[dsl, :]),
            "w3": np.ascontiguousarray(w3_r.astype(NP_BF16)),
            "wro": np.ascontiguousarray(w_ro_t[:, dsl].astype(NP_BF16)),
            "wg": np.ascontiguousarray(wg_r.astype(NP_BF16)),
            "wu": np.ascontiguousarray(wu_r.astype(NP_BF16)),
            "wd": np.ascontiguousarray(wd_r.astype(NP_BF16)),
            "smalls": np.ascontiguousarray(smalls_r.astype(f)),
        })
    return in_maps


def run_on_device(inputs, trace=False, tmpdir=None):
    if "nc" not in _CACHE:
        _CACHE["nc"] = build_nc()
    nc = _CACHE["nc"]
    in_maps = _prep_inputs(inputs)
    res = run_bass_kernel_spmd(nc, in_maps, list(range(NC)),
                               trace=trace, tmpdir=tmpdir)
    shards = [np.asarray(res.results[r]["y"]) for r in range(NC)]
    yt = np.concatenate(shards, axis=0)                  # [D, BT]
    out = np.ascontiguousarray(yt.T).reshape(B, T, D).astype(np.float32)
    return out, res


def kernel(**inputs):
    out, _ = run_on_device(inputs, trace=False)
    return out

